# revision 11
# baseline (speedup 1.0000x reference)
"""HOIContactLoss on Trainium2 — v7: group-folded tree + real-column trim.

vs v6: d2w pair tiles are two persistent ping-pong buffers whose pad
columns [4000:4096] are memset to BIG once and never overwritten, so the
last matmul shrinks to 416 real cols and the B drain to 1952 cols while
the pow2 fold tree stays valid; rminY is initialized by a 4x tensor_copy
of tile 0 instead of memset+min; cham_y transposes run in 2 groups of 16.
"""
import numpy as np
import ml_dtypes

import concourse.bacc as bacc
import concourse.tile as tile
from concourse import mybir
from concourse.bass_utils import run_bass_kernel_spmd
from contextlib import ExitStack

F32, F16, BF16 = mybir.dt.float32, mybir.dt.float16, mybir.dt.bfloat16
AOP = mybir.AluOpType
ACTF = mybir.ActivationFunctionType

B, P1, P2, D = 16, 6890, 4000, 3
P1P, P2P = 6912, 4096          # padded sizes
NT = P1P // 128                # 54 x-tiles of 128 points
BIG = 30000.0                  # "infinity" that stays finite in fp16 even doubled
N_CORES = 8
IPC = B // N_CORES             # items per core

_compiled = None


def _build():
    nc = bacc.Bacc(None, target_bir_lowering=False)
    with tile.TileContext(nc) as tc:
        with ExitStack() as ctx:
            dram = ctx.enter_context(tc.tile_pool(name="dram", bufs=1, space="DRAM"))
            const = ctx.enter_context(tc.tile_pool(name="const", bufs=1))
            io = ctx.enter_context(tc.tile_pool(name="io", bufs=2))
            acc = ctx.enter_context(tc.tile_pool(name="acc", bufs=2))
            foldp = ctx.enter_context(tc.tile_pool(name="foldp", bufs=2))
            ppool = ctx.enter_context(tc.tile_pool(name="ppool", bufs=2, space="PSUM"))

            xf_d = dram.tile([IPC, 13, P1P], BF16, kind="ExternalInput")
            yf_d = dram.tile([IPC, 13, P2P], BF16, kind="ExternalInput")
            chamx_d = dram.tile([IPC, 128, NT, 128], F16, kind="ExternalOutput")
            rminy_d = dram.tile([IPC, 2, 128, P2P], F16, kind="ExternalOutput")
            d2wbufs = []
            for b in range(2):
                d2wb = const.tile([128, 2, P2P], F16, name=f"d2wbuf{b}")
                nc.vector.memset(d2wb[:, :, 4000:4096], BIG)
                d2wbufs.append(d2wb)

            for it in range(IPC):
                yf = io.tile([13, P2P], BF16, tag="yf")
                nc.sync.dma_start(out=yf[:], in_=yf_d[it])
                xfA = io.tile([13, 1024], BF16, tag="xfA")
                nc.sync.dma_start(out=xfA[:], in_=xf_d[it][:, 0:1024])
                xfB = io.tile([13, P1P - 1024], BF16, tag="xfB")
                nc.sync.dma_start(out=xfB[:], in_=xf_d[it][:, 1024:P1P])

                rmA = acc.tile([128, P2P], F16, tag="rmA")
                nc.vector.memset(rmA[:, 4000:4096], BIG)
                rmB = acc.tile([128, P2P], F16, tag="rmB")
                nc.vector.memset(rmB[:, 4000:4096], BIG)
                chamX128 = acc.tile([128, NT, 128], F16, tag="chamX128")

                GROUPS = [8, 8, 8, 8, 8, 8, 6]
                gstart = 0
                pp = 0
                for gi, G in enumerate(GROUPS):
                    f1g = foldp.tile([128, 8, 2048], F16, tag="f1", name=f"f1_{it}_{gi}")
                    for p in range(G // 2):
                        d2w = d2wbufs[pp % 2]
                        pp += 1
                        for k in range(2):
                            t = gstart + 2 * p + k
                            if t < 8:
                                lhsT = xfA[:, t * 128:(t + 1) * 128]
                            else:
                                lhsT = xfB[:, (t - 8) * 128:(t - 7) * 128]
                            pgA = ppool.tile([128, 2048], F32, tag="pg", name=f"pgA_{it}_{t}")
                            for c in range(4):
                                nc.tensor.matmul(pgA[:, c * 512:(c + 1) * 512], lhsT,
                                                 yf[:, c * 512:(c + 1) * 512],
                                                 start=True, stop=True)
                            pgB = ppool.tile([128, 2048], F32, tag="pg", name=f"pgB_{it}_{t}")
                            for c in range(3):
                                nc.tensor.matmul(pgB[:, c * 512:(c + 1) * 512], lhsT,
                                                 yf[:, (c + 4) * 512:(c + 5) * 512],
                                                 start=True, stop=True)
                            nc.tensor.matmul(pgB[:, 1536:1952], lhsT, yf[:, 3584:4000],
                                             start=True, stop=True)
                            nc.scalar.activation(out=d2w[:, k, 0:2048], in_=pgA[:], func=ACTF.Relu)
                            nc.scalar.activation(out=d2w[:, k, 2048:4000], in_=pgB[:, 0:1952], func=ACTF.Relu)
                            # cham_y: even/odd accumulators (rmA final after tile 23)
                            rm = rmA if t < 24 else rmB
                            if t == 0 or t == 24:
                                nc.vector.tensor_copy(out=rm[:, 0:4000], in_=d2w[:, k, 0:4000])
                            else:
                                nc.vector.tensor_tensor(rm[:, 0:4000], d2w[:, k, 0:4000],
                                                        rm[:, 0:4000], op=AOP.min)
                        # fold L1 for both tiles of the pair in one op (pad cols are BIG)
                        nc.vector.tensor_tensor(f1g[:, 2 * p:2 * p + 2, :],
                                                d2w[:, :, 0:2048], d2w[:, :, 2048:4096], op=AOP.min)
                    # grouped fold levels: one op per level for all G tiles
                    fg = f1g[:, 0:G, :]
                    nc.vector.tensor_tensor(fg[:, :, 0:1024], fg[:, :, 0:1024], fg[:, :, 1024:2048], op=AOP.min)
                    nc.vector.tensor_tensor(fg[:, :, 0:512], fg[:, :, 0:512], fg[:, :, 512:1024], op=AOP.min)
                    nc.vector.tensor_tensor(fg[:, :, 0:256], fg[:, :, 0:256], fg[:, :, 256:512], op=AOP.min)
                    nc.vector.tensor_tensor(chamX128[:, gstart:gstart + G, :],
                                            fg[:, :, 0:128], fg[:, :, 128:256], op=AOP.min)
                    nc.scalar.dma_start(out=chamx_d[it][:, gstart:gstart + G, :],
                                        in_=chamX128[:, gstart:gstart + G, :])
                    gstart += G
                    if gstart == 24:
                        nc.sync.dma_start(out=rminy_d[it][0, :, 0:2048], in_=rmA[:, 0:2048])
                        nc.scalar.dma_start(out=rminy_d[it][0, :, 2048:4096], in_=rmA[:, 2048:4096])


                nc.sync.dma_start(out=rminy_d[it][1, :, 0:2048], in_=rmB[:, 0:2048])
                nc.scalar.dma_start(out=rminy_d[it][1, :, 2048:4096], in_=rmB[:, 2048:4096])

            names = dict(xf=xf_d.name, yf=yf_d.name, chamx=chamx_d.name, rminy=rminy_d.name)
    nc.compile()
    return nc, names


def _bf16(a):
    return a.astype(ml_dtypes.bfloat16)


def _prep_item(x, y, sm, om, n):
    """Build lifted-feature tensors for one batch item (host-side repacking)."""
    xx = np.zeros((P1P, 3), np.float32); xx[:P1] = x
    yy = np.zeros((P2P, 3), np.float32); yy[:P2] = y
    x2 = (xx * xx).sum(-1); x2[P1:] = BIG
    y2 = (yy * yy).sum(-1)
    mask = (np.arange(P2P) >= n).astype(np.float32) * BIG
    y2m = y2 + mask
    t = -2.0 * yy
    xh = _bf16(xx); xl = _bf16(xx - xh.astype(np.float32))
    th = _bf16(t);  tl = _bf16(t - th.astype(np.float32))
    x2h = _bf16(x2); x2l = _bf16(x2 - x2h.astype(np.float32))
    y2mh = _bf16(y2m); y2ml = _bf16(y2m - y2mh.astype(np.float32))
    o1 = np.ones(P1P, ml_dtypes.bfloat16); o2 = np.ones(P2P, ml_dtypes.bfloat16)
    XF = np.stack([xh[:, 0], xh[:, 1], xh[:, 2], xl[:, 0], xl[:, 1], xl[:, 2],
                   xh[:, 0], xh[:, 1], xh[:, 2], x2h, x2l, o1, o1])
    YF = np.stack([th[:, 0], th[:, 1], th[:, 2], th[:, 0], th[:, 1], th[:, 2],
                   tl[:, 0], tl[:, 1], tl[:, 2], o2, o2, y2mh, y2ml])
    smp = np.zeros(P1P, np.float32); smp[:P1] = sm[:, 0]
    omp = np.zeros(P2P, np.float32)
    omp[:P2] = np.where(np.arange(P2) < n, om[:, 0], 0.0)
    SM = smp.reshape(NT, 128).T.copy()          # [128, 54] partition-major
    OM = omp.reshape(32, 128).T.copy()          # [128, 32] partition-major
    return XF, YF, SM, OM


def kernel(smpl_v, object_v, smpl_contact_maps, object_contact_maps, object_verts_n,
           trace=False):
    global _compiled
    if _compiled is None:
        _compiled = _build()
    nc, names = _compiled

    smpl_v = np.asarray(smpl_v, np.float32)
    object_v = np.asarray(object_v, np.float32)
    smpl_contact_maps = np.asarray(smpl_contact_maps, np.float32)
    object_contact_maps = np.asarray(object_contact_maps, np.float32)
    ns = np.asarray(object_verts_n).astype(np.int64)

    in_maps, wmaps = [], []
    for c in range(N_CORES):
        XFs, YFs, SMs, OMs = [], [], [], []
        for k in range(IPC):
            b = c * IPC + k
            XF, YF, SM, OM = _prep_item(smpl_v[b], object_v[b], smpl_contact_maps[b],
                                        object_contact_maps[b], int(ns[b]))
            XFs.append(XF); YFs.append(YF); SMs.append(SM); OMs.append(OM)
        in_maps.append({names['xf']: np.stack(XFs), names['yf']: np.stack(YFs)})
        wmaps.append((SMs, OMs))
    res = run_bass_kernel_spmd(nc, in_maps, core_ids=list(range(N_CORES)), trace=trace)
    losses = []
    for c in range(N_CORES):
        cx = np.asarray(res.results[c][names['chamx']], np.float64).min(axis=3)  # [IPC, 128, 54]
        rm = np.asarray(res.results[c][names['rminy']], np.float64)   # [IPC, 2, 128, 4096]
        SMs, OMs = wmaps[c]
        for k in range(IPC):
            SM = np.asarray(SMs[k], np.float64)
            OM = np.asarray(OMs[k], np.float64)
            lx = (cx[k] * SM).sum() / (SM.sum() + 1e-6)
            chamy = rm[k].min(axis=(0, 1))
            omp = OM.T.reshape(-1)
            ly = (omp * chamy).sum() / (omp.sum() + 1e-6)
            losses.append(lx + ly)
    out = np.float32(np.mean(losses))
    if trace:
        return out, res
    return out
